# revision 7
# baseline (speedup 1.0000x reference)
"""Cross-attention Trainium2 kernel (8 NeuronCores, SPMD).

Sharding: data-parallel over B (4 batches x 2 cores/batch), tensor-parallel
over heads (6+6) within each batch pair. Host sums the two partial output
projections per batch (the "all-reduce after proj" done at gather time).

On-chip layout is fully "transposed" (feature-major) so no transposes are
ever needed: qT/kT [d, tok] come straight out of the projections, scores are
computed as s.T[j, i], softmax sums run over partitions via DVE accumulation
+ a ones-matmul, and attention output oT [d, i] feeds the final projection
directly. All matmuls run in float32r (full PE speed at N>=256, ~13x more
accurate than bf16).

walrus in this toolchain accepts at most ONE sync wait per instruction; the
_split_waits post-pass hoists extra waits onto same-engine NoOps.
"""
import numpy as np

import bass_rust
import concourse.bass as bass
import concourse.tile as tile
from concourse import mybir
from concourse.bass_utils import run_bass_kernel_spmd

B, Nx, Ny, DIM, H, D = 4, 1024, 2048, 768, 12, 64
HL = H // 2          # heads per core
DL = HL * D          # 384 local head dims
NCH = DIM // 128     # 6 contraction chunks
NPAIR = HL // 2      # 3 head pairs per core
NJ = Ny // 128       # 16 key tiles
SCALE = D ** -0.5

F32 = mybir.dt.float32
F32R = mybir.dt.float32r
BF16 = mybir.dt.bfloat16
EXP = mybir.ActivationFunctionType.Exp


def _split_waits(nc):
    ctr = 0
    for f in nc.m.functions:
        for blk in f.blocks:
            insts = blk.instructions
            new_list = []
            changed = False
            for inst in insts:
                si = getattr(inst, "sync_info", None)
                waits = list(si.on_wait) if (si and si.on_wait) else []
                if len(waits) > 1:
                    changed = True
                    for w in waits[:-1]:
                        ctr += 1
                        nop = bass_rust.InstNoOp(
                            name=f"WSPLIT-{ctr}",
                            engine=inst.engine,
                            sync_info=mybir.SyncInfo(on_wait=[w], on_update=[]),
                        )
                        nc.register_instruction(nop)
                        new_list.append(nop)
                    inst.sync_info = mybir.SyncInfo(
                        on_wait=[waits[-1]], on_update=list(si.on_update or [])
                    )
                new_list.append(inst)
            if changed:
                blk.instructions = new_list
    return nc


def build():
    nc = bass.Bass()
    xT = nc.declare_dram_parameter("xT", [DIM, Nx], F32, isOutput=False)
    yT = nc.declare_dram_parameter("yT", [DIM, Ny], F32, isOutput=False)
    wq = nc.declare_dram_parameter("wq", [DIM, DL], F32, isOutput=False)
    wk = nc.declare_dram_parameter("wk", [DIM, DL], F32, isOutput=False)
    wv = nc.declare_dram_parameter("wv", [DIM, DL], F32, isOutput=False)
    wp = nc.declare_dram_parameter("wp", [DL, DIM], F32, isOutput=False)
    ones_d = nc.declare_dram_parameter("ones", [128, 128], F32, isOutput=False)
    out = nc.declare_dram_parameter("out", [DIM, Nx], F32, isOutput=True)

    with tile.TileContext(nc) as tc:
        with tc.tile_pool(name="attn", bufs=1) as attn:
            qT = attn.tile([128, NCH // 2, Nx], F32R)      # [dout-chunk, i] x3
            kT = attn.tile([128, NCH // 2, Ny], F32R)
            # v with a ones column appended per head: [v_h0|1|v_h1|1] x pair
            vx = attn.tile([128, NJ, NPAIR * 130], BF16)
            wpr = attn.tile([128, NCH // 2, DIM], F32R)
            oT = attn.tile([128, NCH // 2, Nx], F32R)      # normalized attn out
            onesr = attn.tile([128, 128], F32R)
            for p_ in range(NPAIR):
                for h_ in range(2):
                    nc.vector.memset(vx[:, :, p_ * 130 + h_ * 65 + 64], 1.0)

            # ---- phase 1: load, cast to f32r, q/k/v projections ----
            with tc.tile_pool(name="proj", bufs=1) as proj, \
                 tc.tile_pool(name="stage", bufs=3) as stg, \
                 tc.tile_pool(name="pps", bufs=2, space="PSUM") as pps:
                xr = proj.tile([128, NCH, Nx], F32R)
                yr = proj.tile([128, NCH, Ny], F32R)
                wqr = proj.tile([128, NCH, DL], F32R)
                wkr = proj.tile([128, NCH, DL], F32R)
                wvr = proj.tile([128, NCH, DL], F32R)

                def load1(dst, src, n, act=False):
                    st = stg.tile([128, dst.shape[-1]], F32, tag="st")
                    nc.sync.dma_start(st[:], src[n * 128:(n + 1) * 128, :])
                    if act:
                        nc.scalar.copy(dst[:, n, :], st[:])
                    else:
                        nc.vector.tensor_copy(dst[:, n, :], st[:])

                st = stg.tile([128, 128], F32, tag="sto")
                nc.sync.dma_start(st[:], ones_d[:])
                nc.vector.tensor_copy(onesr[:], st[:])
                for n in range(NCH):
                    load1(xr, xT, n, act=True)
                    load1(wqr, wq, n)
                # qT[dc] = wq[:,dc].T @ xT   (3 chunks of 128 dout)
                for dc in range(3):
                    ps = pps.tile([128, Nx], F32, tag="ps")
                    for n in range(NCH):
                        for h in range(2):
                            nc.tensor.matmul(
                                ps[:, h * 512:(h + 1) * 512],
                                wqr[:, n, dc * 128:(dc + 1) * 128],
                                xr[:, n, h * 512:(h + 1) * 512],
                                start=(n == 0), stop=(n == NCH - 1))
                    nc.vector.tensor_copy(qT[:, dc, :], ps[:])
                for n in range(NCH):
                    load1(yr, yT, n, act=True)
                    load1(wkr, wk, n)
                    load1(wvr, wv, n)
                for n in range(NCH // 2):
                    load1(wpr, wp, n)
                # kT[dc] = wk[:,dc].T @ yT  (split Ny in halves for PSUM)
                for dc in range(3):
                    for yh in range(2):
                        ps = pps.tile([128, Nx], F32, tag="ps")
                        for n in range(NCH):
                            for h in range(2):
                                nc.tensor.matmul(
                                    ps[:, h * 512:(h + 1) * 512],
                                    wkr[:, n, dc * 128:(dc + 1) * 128],
                                    yr[:, n, yh * 1024 + h * 512:yh * 1024 + (h + 1) * 512],
                                    start=(n == 0), stop=(n == NCH - 1))
                        nc.vector.tensor_copy(kT[:, dc, yh * 1024:(yh + 1) * 1024], ps[:])
                # v[t] = yT[:, t].T @ wv   (natural orientation, 16 tok tiles)
                for t in range(NJ):
                    ps = pps.tile([128, DL], F32, tag="psv")
                    for n in range(NCH):
                        nc.tensor.matmul(
                            ps[:], yr[:, n, t * 128:(t + 1) * 128], wvr[:, n, :],
                            start=(n == 0), stop=(n == NCH - 1))
                    for p_ in range(NPAIR):
                        for h_ in range(2):
                            nc.vector.tensor_copy(
                                vx[:, t, p_ * 130 + h_ * 65:p_ * 130 + h_ * 65 + 64],
                                ps[:, p_ * 128 + h_ * 64:p_ * 128 + (h_ + 1) * 64])

            # ---- phase 2: attention, one head-pair at a time ----
            # scores double-buffered in i-halves; AV carries a ones column so
            # softmax denominators fall out of the matmul (row 64).
            with tc.tile_pool(name="work", bufs=2) as work, \
                 tc.tile_pool(name="ssc", bufs=2, space="PSUM") as ssc, \
                 tc.tile_pool(name="sav", bufs=1, space="PSUM") as sav:
                for p in range(NPAIR):
                    o0 = sav.tile([65, Nx], F32, tag="o0")
                    o1 = sav.tile([65, Nx], F32, tag="o1")
                    for j in range(NJ):
                        js = slice(j * 128, (j + 1) * 128)
                        for hf in range(2):
                            cs = slice(hf * 512, (hf + 1) * 512)
                            s_ps = ssc.tile([128, 2, 512], F32, tag="sps")
                            nc.tensor.matmul(
                                s_ps[:, 0, :], kT[0:64, p, js], qT[0:64, p, cs],
                                start=True, stop=True, tile_position=(0, 0))
                            nc.tensor.matmul(
                                s_ps[:, 1, :], kT[64:128, p, js], qT[64:128, p, cs],
                                start=True, stop=True, tile_position=(64, 0))
                            pT = work.tile([128, 2, 512], BF16, tag="pT")
                            nc.scalar.activation(pT[:], s_ps[:], EXP, scale=SCALE)
                            nc.tensor.matmul(
                                o0[:, cs], vx[:, j, p * 130:p * 130 + 65], pT[:, 0, :],
                                start=(j == 0), stop=(j == NJ - 1))
                            nc.tensor.matmul(
                                o1[:, cs], vx[:, j, p * 130 + 65:p * 130 + 130], pT[:, 1, :],
                                start=(j == 0), stop=(j == NJ - 1))
                    for h, ops in ((0, o0), (1, o1)):
                        rS = work.tile([1, Nx], F32R, tag="rS")
                        with nc.allow_low_precision(reason="f32r is fp32-width"):
                            nc.vector.reciprocal(rS[:], ops[64:65, :])
                        bc = ssc.tile([64, Nx], F32, tag="sps")
                        for hf in range(2):
                            cs = slice(hf * 512, (hf + 1) * 512)
                            nc.tensor.matmul(bc[:, cs], onesr[0:1, 0:64], rS[:, cs],
                                             start=True, stop=True)
                        bc_s = work.tile([64, Nx], F32R, tag="bcs")
                        nc.vector.tensor_copy(bc_s[:], bc[:])
                        nc.vector.tensor_mul(oT[h * 64:(h + 1) * 64, p, :],
                                             ops[0:64, :], bc_s[:])

            # ---- phase 3: output projection ----
            with tc.tile_pool(name="outp", bufs=2) as outp, \
                 tc.tile_pool(name="sy", bufs=2, space="PSUM") as sy:
                for e in range(6):
                    y_ps = sy.tile([128, Nx], F32, tag="yps")
                    for pc in range(3):
                        for hf in range(2):
                            cs = slice(hf * 512, (hf + 1) * 512)
                            nc.tensor.matmul(
                                y_ps[:, cs], wpr[:, pc, e * 128:(e + 1) * 128],
                                oT[:, pc, cs], start=(pc == 0), stop=(pc == 2))
                    o_st = outp.tile([128, Nx], F32, tag="ost")
                    nc.vector.tensor_copy(o_st[:], y_ps[:])
                    nc.sync.dma_start(out[e * 128:(e + 1) * 128, :], o_st[:])

    return _split_waits(nc)


_CACHE = {}


def kernel(x, y, Wq, Wk, Wv, Wproj, bproj):
    if "nc" not in _CACHE:
        _CACHE["nc"] = build()
    nc = _CACHE["nc"]
    x = np.asarray(x, np.float32)
    y = np.asarray(y, np.float32)
    ones = np.ones((128, 128), np.float32)
    in_maps = []
    for c in range(8):
        b, hs = c // 2, (c % 2) * HL
        sl = slice(hs * D, (hs + HL) * D)
        in_maps.append({
            "xT": np.ascontiguousarray(x[b].T),
            "yT": np.ascontiguousarray(y[b].T),
            "wq": np.ascontiguousarray(np.asarray(Wq, np.float32)[sl].T),
            "wk": np.ascontiguousarray(np.asarray(Wk, np.float32)[sl].T),
            "wv": np.ascontiguousarray(np.asarray(Wv, np.float32)[sl].T),
            "wp": np.ascontiguousarray(np.asarray(Wproj, np.float32)[:, sl].T),
            "ones": ones,
        })
    r = run_bass_kernel_spmd(nc, in_maps, list(range(8)))
    outv = np.empty((B, Nx, DIM), np.float32)
    bp = np.asarray(bproj, np.float32)
    for b in range(B):
        yt = r.results[2 * b]["out"] + r.results[2 * b + 1]["out"]
        outv[b] = yt.T + bp
    return outv
